# revision 23
# baseline (speedup 1.0000x reference)
"""LoRA linear layer on 8 Trainium2 NeuronCores.

Computes y = x @ W^T + b + 2.0 * (x @ A^T) @ B^T for
x:[4,4096,1024], W:[1024,1024], b:[1024], A:[16,1024], B:[1024,16].

Host side folds the LoRA update into the weight (W_eff = W + 2*B@A, an exact
algebraic identity), so the device kernel is a single GEMM + bias. Sharding is
data-parallel over the 16384 tokens: each of the 8 cores computes a
[2048, 1024] output slice with replicated weights.

Device kernel (per core): y_c[m,o] = sum_d xT_c[d,m] * WeffT[d,o] + b[o]
  - x, Weff, y all bf16 on the wire (rel-err ~3e-3, well inside the 2e-2
    gate), fp32 PSUM accumulation. bf16 streams 1 col/cycle, so the 131072
    streamed columns floor at ~55 us of PE time.
  - The ramp is aggregate-HBM-bound (~255 GB/s effective during queue
    contention), so the schedule minimizes bytes-needed-early: the first
    m-chunk covers 1024 tokens processed as two half-waves over the output
    dim, needing only W-half0 (1 MiB) + x (2 MiB) in the first ~12 us;
    W-half1 and the later chunks' x arrive during the first chunk. Bias
    travels as 2 KiB and is broadcast on-device via a K=1 matmul.
  - Host pre-tiles x/W/y so every DMA granule is contiguous DRAM; granule
    loads alternate between the two HWDGE rings in consumption-deadline
    order.
"""

import numpy as np
import ml_dtypes

import concourse.mybir as mybir
import concourse.tile as tile
from concourse import bacc
from concourse.bass_utils import run_bass_kernel_spmd

N_CORES = 8
P = 128
D = 1024  # in_features (contraction)
O = 1024  # out_features
M_TOTAL = 4 * 4096  # tokens
M = M_TOTAL // N_CORES  # tokens per core
KO = D // P  # k-subtiles
SC = 512  # x tiling granule (tokens)
MT = M // P  # m-tiles per core (16)
SCALING = 2.0

# Set by test harnesses to capture profiling info; harmless otherwise.
TRACE = False
LAST_RESULT = None

_NC_CACHE = None


def _build_nc():
    f32 = mybir.dt.float32
    bf16 = mybir.dt.bfloat16

    nc = bacc.Bacc("TRN2", debug=False)
    # Host-tiled layouts: each leaf [128, 512] block is contiguous in DRAM.
    xT = nc.dram_tensor("xT", [(M // SC) * KO * P, SC], bf16, kind="ExternalInput")
    wT = nc.dram_tensor("wT", [KO * 2 * P, 512], bf16, kind="ExternalInput")
    bias = nc.dram_tensor("bias", [1, O], bf16, kind="ExternalInput")
    y = nc.dram_tensor("y", [MT * 2 * P, 512], bf16, kind="ExternalOutput")

    x_v = xT[:].rearrange("(sc ko p) m -> p sc ko m", ko=KO, p=P)
    w_v = wT[:].rearrange("(ko h p) o -> p ko h o", h=2, p=P)
    y_v = y[:].rearrange("(mt h p) o -> p mt h o", h=2, p=P)

    with tile.TileContext(nc) as tc:
        with (
            tc.tile_pool(name="wpool", bufs=1) as wpool,
            tc.tile_pool(name="bpool", bufs=1) as bpool,
            tc.tile_pool(name="x0pool", bufs=8) as x0pool,
            tc.tile_pool(name="xpool", bufs=2) as xpool,
            tc.tile_pool(name="opool", bufs=10) as opool,
            tc.tile_pool(name="psum", bufs=8, space="PSUM") as psum,
        ):
            wt8 = wpool.tile([P, KO * 2 * 512], bf16, tag="w")
            wt8_v = wt8[:].rearrange("p (ko h o) -> p ko h o", ko=KO, h=2)

            def wslice(ko, half):
                lo = (ko * 2 + half) * 512
                return wt8[:, lo : lo + 512]

            def wload_h0(ko, eng):
                eng.dma_start(wt8_v[:, ko, 0, :], w_v[:, ko, 0, :])

            # chunk-0 x granules: [128, 1024] per ko covering m-tiles 0..7
            # (two contiguous 128 KiB blocks in DRAM, one DMA each)
            xc0 = [None] * KO

            def xc0load(ko, eng):
                t = x0pool.tile([P, 2 * SC], bf16, tag="x0", name=f"xc0_{ko}")
                eng.dma_start(
                    t[:].rearrange("p (s m) -> p s m", s=2), x_v[:, 0:2, ko, :]
                )
                xc0[ko] = t

            # Granule loads alternate rings in consumption-deadline order:
            # round ko of the first half-wave reads W(ko,h0) and x(ko).
            bsm = bpool.tile([1, O], bf16, tag="bsm")
            wload_h0(0, nc.sync)
            xc0load(0, nc.scalar)
            zt = bpool.tile([P, P], bf16, tag="warm")
            nc.gpsimd.memset(zt[:], 0.0)
            # bias (2 KiB) rides the SWDGE ring right behind the warmup
            # memset: lands before the broadcast matmuls read it
            nc.gpsimd.dma_start(bsm[:], bias[:])
            ones = bpool.tile([1, P], bf16, tag="ones")
            nc.gpsimd.memset(ones[:], 1.0)

            # PE warmup: N=128 matmuls on a zeroed tile, long enough that,
            # together with the broadcast matmuls and the first (briefly
            # cold) real matmuls, the HAM clock-gate window (~3.4 us of
            # sustained PE activity) completes just as real data lands.
            wps = psum.tile([P, 512], mybir.dt.float32, tag="ps", name="wps")
            for _ in range(28):
                nc.tensor.matmul(wps[:, :P], zt[:], zt[:], start=True, stop=True)

            xc0load(1, nc.sync)
            wload_h0(1, nc.scalar)
            wload_h0(2, nc.sync)
            xc0load(2, nc.scalar)
            xc0load(3, nc.sync)
            wload_h0(3, nc.scalar)
            wload_h0(4, nc.sync)
            xc0load(4, nc.scalar)
            xc0load(5, nc.sync)
            wload_h0(5, nc.scalar)
            wload_h0(6, nc.sync)
            xc0load(6, nc.scalar)
            xc0load(7, nc.sync)
            wload_h0(7, nc.scalar)
            # W half-1 (needed from the second half-wave, ~14 us after t0)
            # and the later chunks' x ride behind the ramp granules.
            nc.sync.dma_start(wt8_v[:, :, 1, :], w_v[:, :, 1, :])
            xts = {}

            def load_x(sc, eng):
                t = xpool.tile([P, KO * SC], bf16, tag="xt", name=f"x{sc}")
                eng.dma_start(
                    t[:].rearrange("p (ko m) -> p ko m", ko=KO),
                    x_v[:, sc, :, :],
                )
                xts[sc] = t

            load_x(2, nc.scalar)
            load_x(3, nc.scalar)

            bt = bpool.tile([P, O], bf16, tag="bias")

            def bias_broadcast():
                # [128, O] = ones[1,128]^T @ bias[1,O] via two K=1 matmuls
                # into PSUM, evicted to SBUF by the DVE. Emitted inline in
                # chunk0's first ko-round (just before the m-tile whose PSUM
                # group reuses wps's bank) so it never gates the stream
                # start: by then the 2 KiB bias DMA has long landed.
                for half in range(2):
                    nc.tensor.matmul(
                        wps[:],
                        ones[:],
                        bsm[:, half * 512 : (half + 1) * 512],
                        start=True,
                        stop=True,
                    )
                    nc.vector.tensor_copy(
                        bt[:, half * 512 : (half + 1) * 512], wps[:]
                    )

            def x_slice(chunk, ko, mt_i):
                if chunk == 0:
                    return xc0[ko][:, mt_i * P : (mt_i + 1) * P]
                t = xts[chunk + 1]
                lo = ko * SC + mt_i * P
                return t[:, lo : lo + P]

            def evict(ps, ot, half, n0=0, n1=512):
                nc.vector.tensor_tensor(
                    ot[:, n0:n1],
                    ps[:, n0:n1],
                    bt[:, half * 512 + n0 : half * 512 + n1],
                    mybir.AluOpType.add,
                )

            # Chunk 0 half 0 (m-tiles 0..7): ko-outer over eight PSUM groups —
            # the only wave that consumes granules as they stream in, so each
            # ko-round waits on at most one 128 KiB W slice + one 256 KiB x
            # granule. Its evictions spread over the next wave's span.
            pss = [
                psum.tile([P, 512], mybir.dt.float32, tag="ps", name=f"c0h0_{i}")
                for i in range(8)
            ]
            for ko in range(KO):
                for mt_i in range(8):
                    if ko == 0 and mt_i == 7:
                        bias_broadcast()
                    nc.tensor.matmul(
                        pss[mt_i][:],
                        x_slice(0, ko, mt_i),
                        wslice(ko, 0),
                        start=ko == 0,
                        stop=ko == KO - 1,
                    )
            for mt_i in range(8):
                ot = opool.tile([P, 512], bf16, tag="ot", name=f"oc0h0_{mt_i}")
                evict(pss[mt_i], ot, 0)
                nc.gpsimd.dma_start(y_v[:, mt_i, 0, :], ot[:])

            # All later waves run mt-outer: one PSUM group per 8-matmul
            # burst, demanding a freshly-evicted bank only every ~1.7 us —
            # comfortably under the DVE's 0.7 us/bank eviction rate, so no
            # wave boundary ever starves the PE (a ko-outer wave boundary
            # demands all 8 banks in 1.7 us and stalls ~4 us). Their x
            # granules are fully resident by construction.
            # Chunk 0 half 1 (m-tiles 0..7):
            for mt_i in range(8):
                ps = psum.tile(
                    [P, 512], mybir.dt.float32, tag="ps", name=f"c0h1_{mt_i}"
                )
                for ko in range(KO):
                    nc.tensor.matmul(
                        ps[:],
                        x_slice(0, ko, mt_i),
                        wslice(ko, 1),
                        start=ko == 0,
                        stop=ko == KO - 1,
                    )
                ot = opool.tile([P, 512], bf16, tag="ot", name=f"oc0h1_{mt_i}")
                evict(ps, ot, 1)
                nc.gpsimd.dma_start(y_v[:, mt_i, 1, :], ot[:])

            # Chunk 1 (m-tiles 8..11), mt-outer, both halves per m-tile:
            for mt_i in range(4):
                for half in range(2):
                    ps = psum.tile(
                        [P, 512], mybir.dt.float32, tag="ps", name=f"c1_{mt_i}_{half}"
                    )
                    for ko in range(KO):
                        nc.tensor.matmul(
                            ps[:],
                            x_slice(1, ko, mt_i),
                            wslice(ko, half),
                            start=ko == 0,
                            stop=ko == KO - 1,
                        )
                    ot = opool.tile(
                        [P, 512], bf16, tag="ot", name=f"oc1_{mt_i}_{half}"
                    )
                    evict(ps, ot, half)
                    nc.gpsimd.dma_start(y_v[:, 8 + mt_i, half, :], ot[:])

            # Chunk 2 (m-tiles 12..15): mt-outer so evictions and stores
            # spread across its span; the final m-tile ends in two 256-wide
            # quarter groups so the closing eviction+store chain is short.
            for mt_i in range(4):
                mt = 12 + mt_i
                final = mt_i == 3
                if not final:
                    ph = [
                        psum.tile([P, 512], mybir.dt.float32, tag="ps", name=f"c2_{h}")
                        for h in range(2)
                    ]
                    for ko in range(KO):
                        for half in range(2):
                            nc.tensor.matmul(
                                ph[half][:],
                                x_slice(2, ko, mt_i),
                                wslice(ko, half),
                                start=ko == 0,
                                stop=ko == KO - 1,
                            )
                    for half in range(2):
                        ot = opool.tile(
                            [P, 512], bf16, tag="ot", name=f"oc2_{mt_i}_{half}"
                        )
                        evict(ph[half], ot, half)
                        nc.sync.dma_start(y_v[:, mt, half, :], ot[:])
                else:
                    ph0 = psum.tile([P, 512], mybir.dt.float32, tag="ps", name="pf0")
                    for ko in range(KO):
                        nc.tensor.matmul(
                            ph0[:],
                            x_slice(2, ko, mt_i),
                            wslice(ko, 0),
                            start=ko == 0,
                            stop=ko == KO - 1,
                        )
                    ot0 = opool.tile([P, 512], bf16, tag="ot", name="otf0")
                    evict(ph0, ot0, 0)
                    nc.sync.dma_start(y_v[:, mt, 0, :], ot0[:])
                    pq = [
                        psum.tile([P, 512], mybir.dt.float32, tag="ps", name=f"pq{q}")
                        for q in range(2)
                    ]
                    for q in range(2):
                        for ko in range(KO):
                            nc.tensor.matmul(
                                pq[q][:, 0:256],
                                x_slice(2, ko, mt_i),
                                wslice(ko, 1)[:, q * 256 : (q + 1) * 256],
                                start=ko == 0,
                                stop=ko == KO - 1,
                            )
                    otq = opool.tile([P, 512], bf16, tag="ot", name="otq")
                    for q in range(2):
                        nc.vector.tensor_tensor(
                            otq[:, q * 256 : (q + 1) * 256],
                            pq[q][:, 0:256],
                            bt[:, 512 + q * 256 : 512 + (q + 1) * 256],
                            mybir.AluOpType.add,
                        )
                        (nc.sync if q == 0 else nc.scalar).dma_start(
                            y_v[:, mt, 1, q * 256 : (q + 1) * 256],
                            otq[:, q * 256 : (q + 1) * 256],
                        )

    nc.compile()
    return nc


def _get_nc():
    global _NC_CACHE
    if _NC_CACHE is None:
        _NC_CACHE = _build_nc()
    return _NC_CACHE


def kernel(x, W, b, A, B):
    global LAST_RESULT
    x = np.ascontiguousarray(np.asarray(x, dtype=np.float32))
    W = np.asarray(W, dtype=np.float32)
    b = np.asarray(b, dtype=np.float32)
    A = np.asarray(A, dtype=np.float32)
    B = np.asarray(B, dtype=np.float32)
    assert x.shape == (4, 4096, D) and W.shape == (O, D)
    assert b.shape == (O,) and A.shape[1] == D and B.shape[0] == O

    # Fold the LoRA update into the weight: x@W^T + s*(x@A^T)@B^T = x@(W + s*B@A)^T
    Weff = (
        W.astype(np.float64) + SCALING * (B.astype(np.float64) @ A.astype(np.float64))
    ).astype(np.float32)
    WeffT = Weff.T.astype(ml_dtypes.bfloat16)  # [D, O]
    # [KO, P, 2, 512] -> [KO, 2, P, 512]: leaf blocks contiguous per (ko, half)
    w_tiled = np.ascontiguousarray(
        WeffT.reshape(KO, P, 2, 512).transpose(0, 2, 1, 3)
    ).reshape(KO * 2 * P, 512)
    b_sm = np.ascontiguousarray(b[None, :].astype(ml_dtypes.bfloat16))

    n_sc = M // SC
    xr = x.reshape(M_TOTAL, D).astype(ml_dtypes.bfloat16)
    in_maps = []
    for c in range(N_CORES):
        xc = xr[c * M : (c + 1) * M]  # [M, D]
        # x_t[sc, ko, p, j] = xc[sc*512 + j, ko*128 + p]
        x_tiled = np.ascontiguousarray(
            xc.reshape(n_sc, SC, KO, P).transpose(0, 2, 3, 1)
        ).reshape(n_sc * KO * P, SC)
        in_maps.append({"xT": x_tiled, "wT": w_tiled, "bias": b_sm})

    nc = _get_nc()
    res = run_bass_kernel_spmd(
        nc, in_maps, core_ids=list(range(N_CORES)), trace=TRACE
    )
    LAST_RESULT = res

    outs = []
    for c in range(N_CORES):
        y_t = np.asarray(res.results[c]["y"]).reshape(MT, 2, P, 512)
        outs.append(y_t.transpose(0, 2, 1, 3).reshape(M, O))
    out = np.concatenate(outs, axis=0)
    return out.astype(np.float32).reshape(x.shape[0], x.shape[1], O)


# revision 25
# speedup vs baseline: 1.0595x; 1.0595x over previous
"""LoRA linear layer on 8 Trainium2 NeuronCores.

Computes y = x @ W^T + b + 2.0 * (x @ A^T) @ B^T for
x:[4,4096,1024], W:[1024,1024], b:[1024], A:[16,1024], B:[1024,16].

Host side folds the LoRA update into the weight (W_eff = W + 2*B@A, an exact
algebraic identity), so the device kernel is a single GEMM + bias. Sharding is
data-parallel over the 16384 tokens: each of the 8 cores computes a
[2048, 1024] output slice with replicated weights.

Device kernel (per core): y_c[m,o] = sum_d xT_c[d,m] * WeffT[d,o] + b[o]
  - x, Weff, y all bf16 on the wire (rel-err ~3e-3, well inside the 2e-2
    gate), fp32 PSUM accumulation. bf16 streams 1 col/cycle, so the 131072
    streamed columns floor at ~55 us of PE time.
  - The ramp is aggregate-HBM-bound (~255 GB/s effective during queue
    contention), so the schedule minimizes bytes-needed-early: the first
    m-chunk covers 1024 tokens processed as two half-waves over the output
    dim, needing only W-half0 (1 MiB) + x (2 MiB) in the first ~12 us;
    W-half1 and the later chunks' x arrive during the first chunk. Bias
    travels as 2 KiB and is broadcast on-device via a K=1 matmul.
  - Host pre-tiles x/W/y so every DMA granule is contiguous DRAM; granule
    loads alternate between the two HWDGE rings in consumption-deadline
    order.
"""

import numpy as np
import ml_dtypes

import concourse.mybir as mybir
import concourse.tile as tile
from concourse import bacc
from concourse.bass_utils import run_bass_kernel_spmd

N_CORES = 8
P = 128
D = 1024  # in_features (contraction)
O = 1024  # out_features
M_TOTAL = 4 * 4096  # tokens
M = M_TOTAL // N_CORES  # tokens per core
KO = D // P  # k-subtiles
SC = 512  # x tiling granule (tokens)
MT = M // P  # m-tiles per core (16)
SCALING = 2.0

# Set by test harnesses to capture profiling info; harmless otherwise.
TRACE = False
LAST_RESULT = None

_NC_CACHE = None


def _build_nc():
    f32 = mybir.dt.float32
    bf16 = mybir.dt.bfloat16

    nc = bacc.Bacc("TRN2", debug=False)
    # Host-tiled layouts: each leaf [128, 512] block is contiguous in DRAM.
    xT = nc.dram_tensor("xT", [(M // SC) * KO * P, SC], bf16, kind="ExternalInput")
    wT = nc.dram_tensor("wT", [KO * 2 * P, 512], bf16, kind="ExternalInput")
    bias = nc.dram_tensor("bias", [1, O], bf16, kind="ExternalInput")
    y = nc.dram_tensor("y", [MT * 2 * P, 512], bf16, kind="ExternalOutput")

    x_v = xT[:].rearrange("(sc ko p) m -> p sc ko m", ko=KO, p=P)
    w_v = wT[:].rearrange("(ko h p) o -> p ko h o", h=2, p=P)
    y_v = y[:].rearrange("(mt h p) o -> p mt h o", h=2, p=P)

    with tile.TileContext(nc) as tc:
        with (
            tc.tile_pool(name="wpool", bufs=1) as wpool,
            tc.tile_pool(name="bpool", bufs=1) as bpool,
            tc.tile_pool(name="x0pool", bufs=8) as x0pool,
            tc.tile_pool(name="xpool", bufs=2) as xpool,
            tc.tile_pool(name="opool", bufs=10) as opool,
            tc.tile_pool(name="psum", bufs=8, space="PSUM") as psum,
        ):
            wt8 = wpool.tile([P, KO * 2 * 512], bf16, tag="w")
            wt8_v = wt8[:].rearrange("p (ko h o) -> p ko h o", ko=KO, h=2)

            def wslice(ko, half):
                lo = (ko * 2 + half) * 512
                return wt8[:, lo : lo + 512]

            def wload_h0(ko, eng):
                eng.dma_start(wt8_v[:, ko, 0, :], w_v[:, ko, 0, :])

            # chunk-0 x granules: [128, 1024] per ko covering m-tiles 0..7
            # (two contiguous 128 KiB blocks in DRAM, one DMA each)
            xc0 = [None] * KO

            def xc0load(ko, eng):
                t = x0pool.tile([P, 2 * SC], bf16, tag="x0", name=f"xc0_{ko}")
                eng.dma_start(
                    t[:].rearrange("p (s m) -> p s m", s=2), x_v[:, 0:2, ko, :]
                )
                xc0[ko] = t

            # Granule loads alternate rings in consumption-deadline order:
            # round ko of the first half-wave reads W(ko,h0) and x(ko).
            bsm = bpool.tile([1, O], bf16, tag="bsm")
            wload_h0(0, nc.sync)
            xc0load(0, nc.scalar)
            zt = bpool.tile([P, P], bf16, tag="warm")
            nc.gpsimd.memset(zt[:], 0.0)
            # bias (2 KiB) rides the SWDGE ring right behind the warmup
            # memset: lands before the broadcast matmuls read it
            nc.gpsimd.dma_start(bsm[:], bias[:])
            ones = bpool.tile([1, P], bf16, tag="ones")
            nc.gpsimd.memset(ones[:], 1.0)

            # PE warmup: N=128 matmuls on a zeroed tile, long enough that,
            # together with the broadcast matmuls and the first (briefly
            # cold) real matmuls, the HAM clock-gate window (~3.4 us of
            # sustained PE activity) completes just as real data lands.
            wps = psum.tile([P, 512], mybir.dt.float32, tag="ps", name="wps")
            for _ in range(28):
                nc.tensor.matmul(wps[:, :P], zt[:], zt[:], start=True, stop=True)

            xc0load(1, nc.sync)
            wload_h0(1, nc.scalar)
            wload_h0(2, nc.sync)
            xc0load(2, nc.scalar)
            xc0load(3, nc.sync)
            wload_h0(3, nc.scalar)
            wload_h0(4, nc.sync)
            xc0load(4, nc.scalar)
            xc0load(5, nc.sync)
            wload_h0(5, nc.scalar)
            wload_h0(6, nc.sync)
            xc0load(6, nc.scalar)
            xc0load(7, nc.sync)
            wload_h0(7, nc.scalar)
            # The bulk loads ride behind the ramp granules, ordered by
            # consumption deadline and split across both rings so no single
            # late transfer can stall a wave: chunk-1 x (needed t0+14),
            # chunk-2 x (t0+21), then W half-1 (t0+28 — all half-0 waves
            # across all chunks run before any half-1 work).
            xts = {}

            def load_x_half(sc, k0, k1, eng):
                t = xts.get(sc)
                if t is None:
                    t = xpool.tile([P, KO * SC], bf16, tag="xt", name=f"x{sc}")
                    xts[sc] = t
                eng.dma_start(
                    t[:, k0 * SC : k1 * SC].rearrange(
                        "p (ko m) -> p ko m", ko=k1 - k0
                    ),
                    x_v[:, sc, k0:k1, :],
                )

            load_x_half(2, 0, 4, nc.sync)
            load_x_half(2, 4, 8, nc.scalar)
            load_x_half(3, 0, 4, nc.sync)
            load_x_half(3, 4, 8, nc.scalar)
            nc.sync.dma_start(wt8_v[:, 0:4, 1, :], w_v[:, 0:4, 1, :])
            nc.scalar.dma_start(wt8_v[:, 4:8, 1, :], w_v[:, 4:8, 1, :])

            bt = bpool.tile([P, O], bf16, tag="bias")

            def bias_broadcast():
                # [128, O] = ones[1,128]^T @ bias[1,O] via two K=1 matmuls
                # into PSUM, evicted to SBUF by the DVE. Emitted inline in
                # chunk0's first ko-round (just before the m-tile whose PSUM
                # group reuses wps's bank) so it never gates the stream
                # start: by then the 2 KiB bias DMA has long landed.
                for half in range(2):
                    nc.tensor.matmul(
                        wps[:],
                        ones[:],
                        bsm[:, half * 512 : (half + 1) * 512],
                        start=True,
                        stop=True,
                    )
                    nc.vector.tensor_copy(
                        bt[:, half * 512 : (half + 1) * 512], wps[:]
                    )

            def x_slice(chunk, ko, mt_i):
                if chunk == 0:
                    return xc0[ko][:, mt_i * P : (mt_i + 1) * P]
                t = xts[chunk + 1]
                lo = ko * SC + mt_i * P
                return t[:, lo : lo + P]

            def evict(ps, ot, half, n0=0, n1=512):
                nc.vector.tensor_tensor(
                    ot[:, n0:n1],
                    ps[:, n0:n1],
                    bt[:, half * 512 + n0 : half * 512 + n1],
                    mybir.AluOpType.add,
                )

            # Chunk 0 half 0 (m-tiles 0..7): ko-outer over eight PSUM groups —
            # the only wave that consumes granules as they stream in, so each
            # ko-round waits on at most one 128 KiB W slice + one 256 KiB x
            # granule. Its evictions spread over the next wave's span.
            pss = [
                psum.tile([P, 512], mybir.dt.float32, tag="ps", name=f"c0h0_{i}")
                for i in range(8)
            ]
            for ko in range(KO):
                for mt_i in range(8):
                    if ko == 0 and mt_i == 7:
                        bias_broadcast()
                    nc.tensor.matmul(
                        pss[mt_i][:],
                        x_slice(0, ko, mt_i),
                        wslice(ko, 0),
                        start=ko == 0,
                        stop=ko == KO - 1,
                    )
            for mt_i in range(8):
                ot = opool.tile([P, 512], bf16, tag="ot", name=f"oc0h0_{mt_i}")
                evict(pss[mt_i], ot, 0)
                nc.gpsimd.dma_start(y_v[:, mt_i, 0, :], ot[:])

            # All remaining waves run mt-outer: one PSUM group per 8-matmul
            # burst, demanding a freshly-evicted bank only every ~1.7 us —
            # comfortably under the DVE's 0.7 us/bank eviction rate, so no
            # wave boundary ever starves the PE (a ko-outer wave boundary
            # demands all 8 banks in 1.7 us and stalls ~4 us).
            # Half-0 for chunks 1 and 2 first (uses only resident W half-0
            # while W half-1 and later x still stream in), then half-1 for
            # chunks 0..2. The very last m-tile ends in two 256-wide quarter
            # groups so the closing eviction+store chain is short.
            def mt_wave(chunk, mt_i, mt, half):
                ps = psum.tile(
                    [P, 512], mybir.dt.float32, tag="ps", name=f"w{chunk}_{mt_i}_{half}"
                )
                for ko in range(KO):
                    nc.tensor.matmul(
                        ps[:],
                        x_slice(chunk, ko, mt_i),
                        wslice(ko, half),
                        start=ko == 0,
                        stop=ko == KO - 1,
                    )
                ot = opool.tile(
                    [P, 512], bf16, tag="ot", name=f"o{chunk}_{mt_i}_{half}"
                )
                evict(ps, ot, half)
                nc.gpsimd.dma_start(y_v[:, mt, half, :], ot[:])

            for mt_i in range(4):
                mt_wave(1, mt_i, 8 + mt_i, 0)
            for mt_i in range(4):
                mt_wave(2, mt_i, 12 + mt_i, 0)
            for mt_i in range(8):
                mt_wave(0, mt_i, mt_i, 1)
            for mt_i in range(4):
                mt_wave(1, mt_i, 8 + mt_i, 1)
            for mt_i in range(3):
                mt_wave(2, mt_i, 12 + mt_i, 1)

            # final m-tile (mt 15, half 1): two 256-wide quarter groups
            mt = 15
            pq = [
                psum.tile([P, 512], mybir.dt.float32, tag="ps", name=f"pq{q}")
                for q in range(2)
            ]
            for q in range(2):
                for ko in range(KO):
                    nc.tensor.matmul(
                        pq[q][:, 0:256],
                        x_slice(2, ko, 3),
                        wslice(ko, 1)[:, q * 256 : (q + 1) * 256],
                        start=ko == 0,
                        stop=ko == KO - 1,
                    )
            otq = opool.tile([P, 512], bf16, tag="ot", name="otq")
            for q in range(2):
                nc.vector.tensor_tensor(
                    otq[:, q * 256 : (q + 1) * 256],
                    pq[q][:, 0:256],
                    bt[:, 512 + q * 256 : 512 + (q + 1) * 256],
                    mybir.AluOpType.add,
                )
                (nc.sync if q == 0 else nc.scalar).dma_start(
                    y_v[:, mt, 1, q * 256 : (q + 1) * 256],
                    otq[:, q * 256 : (q + 1) * 256],
                )

    nc.compile()
    return nc


def _get_nc():
    global _NC_CACHE
    if _NC_CACHE is None:
        _NC_CACHE = _build_nc()
    return _NC_CACHE


def kernel(x, W, b, A, B):
    global LAST_RESULT
    x = np.ascontiguousarray(np.asarray(x, dtype=np.float32))
    W = np.asarray(W, dtype=np.float32)
    b = np.asarray(b, dtype=np.float32)
    A = np.asarray(A, dtype=np.float32)
    B = np.asarray(B, dtype=np.float32)
    assert x.shape == (4, 4096, D) and W.shape == (O, D)
    assert b.shape == (O,) and A.shape[1] == D and B.shape[0] == O

    # Fold the LoRA update into the weight: x@W^T + s*(x@A^T)@B^T = x@(W + s*B@A)^T
    Weff = (
        W.astype(np.float64) + SCALING * (B.astype(np.float64) @ A.astype(np.float64))
    ).astype(np.float32)
    WeffT = Weff.T.astype(ml_dtypes.bfloat16)  # [D, O]
    # [KO, P, 2, 512] -> [KO, 2, P, 512]: leaf blocks contiguous per (ko, half)
    w_tiled = np.ascontiguousarray(
        WeffT.reshape(KO, P, 2, 512).transpose(0, 2, 1, 3)
    ).reshape(KO * 2 * P, 512)
    b_sm = np.ascontiguousarray(b[None, :].astype(ml_dtypes.bfloat16))

    n_sc = M // SC
    xr = x.reshape(M_TOTAL, D).astype(ml_dtypes.bfloat16)
    in_maps = []
    for c in range(N_CORES):
        xc = xr[c * M : (c + 1) * M]  # [M, D]
        # x_t[sc, ko, p, j] = xc[sc*512 + j, ko*128 + p]
        x_tiled = np.ascontiguousarray(
            xc.reshape(n_sc, SC, KO, P).transpose(0, 2, 3, 1)
        ).reshape(n_sc * KO * P, SC)
        in_maps.append({"xT": x_tiled, "wT": w_tiled, "bias": b_sm})

    nc = _get_nc()
    res = run_bass_kernel_spmd(
        nc, in_maps, core_ids=list(range(N_CORES)), trace=TRACE
    )
    LAST_RESULT = res

    outs = []
    for c in range(N_CORES):
        y_t = np.asarray(res.results[c]["y"]).reshape(MT, 2, P, 512)
        outs.append(y_t.transpose(0, 2, 1, 3).reshape(M, O))
    out = np.concatenate(outs, axis=0)
    return out.astype(np.float32).reshape(x.shape[0], x.shape[1], O)
